# revision 32
# baseline (speedup 1.0000x reference)
"""Gated Linear Attention forward on 8 Trainium2 NeuronCores (Bass/Tile).

Problem: B=4, T=1024, D=1024, H=8, DK=64, DV=128, conv4 on q/k/v, low-rank
log-sigmoid forget gate, recurrent scan, RMS-norm + swish output gate, out proj.

Sharding: core = 2*b + hg  (b = batch, hg = half of the heads).
Each core computes its batch's tokens for 4 heads end-to-end and a partial
output projection (Wo row-block); the host sums the two partials per batch.

On-device algorithm: chunked-parallel GLA with chunk C=128.
Per chunk (local inclusive cumsum b of the log-gates):
  q~ = q * exp(b)/8,  k~ = k * exp(-b)
  A~[s,t] = sum_kk k~[s] q~[t]   masked to s<=t
  o = A~^T v (intra) + q~ @ S (inter), accumulated in one PSUM tile
  S' = diag(exp(b_C)) (S + k~^T v)

v3 engine notes (informed by HW traces):
- All matmul operands bf16 (PSUM fp32); gk/cumsum fp32. Host pre-packs
  every input into [128, ...] partition-major bf16 slabs (incl. the conv
  diag matrices) so each is one simple 2D HWDGE DMA; src lands first split
  across the 3 DMA-capable queues.
- The GpSimd engine only does memsets + output DMA (its software tensor
  ops measure ~2.4us each - never on the critical path).
- ACT table switches are expensive; the chunk phase uses only
  Copy/Square/Sqrt (one table): rrms = DVE-reciprocal(ACT-Sqrt(ms+eps)).
- Small per-(c,g) ops are batched full-width: one 512-wide PSUM bank per
  step holds both head groups (A~, o, k^T v), so DVE/ACT run 1 wide op
  instead of 2-4 narrow ones.
- The output projection is pipelined per chunk inside the recurrence loop
  so the PE never drains; output DMA (bf16) overlaps the chunk compute.
"""

import numpy as np
import ml_dtypes

import concourse.bass as bass
import concourse.mybir as mybir
import concourse.tile as tile
from concourse import bacc
from concourse.bass_utils import run_bass_kernel_spmd

F32 = mybir.dt.float32
BF16 = mybir.dt.bfloat16
AF = mybir.ActivationFunctionType
OP = mybir.AluOpType

# problem constants (hardcoded per the task contract)
B, T, D, H = 4, 1024, 1024, 8
KD, VD = 512, 1024
DK, DV = 64, 128
CONV = 4
GATE_NORM = 16.0
EPS = 1e-5
LN8 = float(np.log(8.0))

# per-core shapes
KDC, VDC = 256, 512          # q/k and v/gate channels per core
MIQ, MIV = 2, 4              # 128-wide channel tiles for q/k and v
C, NCH = 128, 8              # chunk length, number of chunks
G = 2                        # head groups of 2 heads (128 chans) per core
NCORES = 8
NDIAG = (2 * MIQ + MIV) * CONV   # 32 diag matrices

BF = ml_dtypes.bfloat16


def build_program():
    nc = bacc.Bacc("TRN2", target_bir_lowering=False, debug=False)

    # ---- DRAM I/O (all pre-packed partition-major on the host) -------------
    srcT_d = nc.dram_tensor("srcT_in", [128, 8 * T], BF16, kind="ExternalInput")
    wq_d = nc.dram_tensor("wq", [128, 8 * KDC], BF16, kind="ExternalInput")
    wk_d = nc.dram_tensor("wk", [128, 8 * KDC], BF16, kind="ExternalInput")
    wv_d = nc.dram_tensor("wv", [128, 8 * VDC], BF16, kind="ExternalInput")
    wgate_d = nc.dram_tensor("wgate", [128, 8 * VDC], BF16, kind="ExternalInput")
    weff_d = nc.dram_tensor("weff", [128, 8 * KDC], BF16, kind="ExternalInput")
    bgn_d = nc.dram_tensor("bgn", [128, MIQ], F32, kind="ExternalInput")
    wo_d = nc.dram_tensor("wo", [128, 4 * D], BF16, kind="ExternalInput")
    convdiag_d = nc.dram_tensor("convdiag", [128, NDIAG * 128], BF16,
                                kind="ExternalInput")
    maskc_d = nc.dram_tensor("maskc", [128, NCH], F32, kind="ExternalInput")
    out_d = nc.dram_tensor("out", [T, D], BF16, kind="ExternalOutput")

    ident_np = np.eye(128, dtype=np.float32).astype(BF)
    u = np.triu(np.ones((128, 128), np.float32))  # U[s,t] = 1 iff s <= t
    triu4_np = np.concatenate([u, u, u, u], axis=1).astype(BF)
    ident_d = nc.inline_tensor(ident_np, "ident_c")
    triu4_d = nc.inline_tensor(triu4_np, "triu4_c")

    # ---- static SBUF -------------------------------------------------------
    srcT = nc.alloc_sbuf_tensor("srcT", [128, 8, T], BF16)      # src^T, d-major
    wq_sb = nc.alloc_sbuf_tensor("wq_sb", [128, 8, KDC], BF16)
    wk_sb = nc.alloc_sbuf_tensor("wk_sb", [128, 8, KDC], BF16)
    wv_sb = nc.alloc_sbuf_tensor("wv_sb", [128, 8, VDC], BF16)
    wgate_sb = nc.alloc_sbuf_tensor("wgate_sb", [128, 8, VDC], BF16)
    weff_sb = nc.alloc_sbuf_tensor("weff_sb", [128, 8, KDC], BF16)
    bgn_sb = nc.alloc_sbuf_tensor("bgn_sb", [128, MIQ], F32)
    wo_sb = nc.alloc_sbuf_tensor("wo_sb", [128, 4, D], BF16)
    dgall = nc.alloc_sbuf_tensor("dgall", [128, NDIAG, 128], BF16)
    maskc_sb = nc.alloc_sbuf_tensor("maskc_sb", [128, NCH], F32)
    ident = nc.alloc_sbuf_tensor("ident", [128, 128], BF16)
    triu4 = nc.alloc_sbuf_tensor("triu4", [128, 512], BF16)
    ones_sb = nc.alloc_sbuf_tensor("ones_sb", [128, 128], F32)

    enx = nc.alloc_sbuf_tensor("enx", [128, MIQ, T], F32)       # exp(-logit)
    spT = nc.alloc_sbuf_tensor("spT", [128, MIQ, T], F32)       # softplus(-gk_logit)
    bsum = nc.alloc_sbuf_tensor("bsum", [128, MIQ, T], F32)     # per-chunk cumsum
    Eall = nc.alloc_sbuf_tensor("Eall", [128, MIQ, NCH], F32)   # exp(b_C) per chunk
    texq = nc.alloc_sbuf_tensor("texq", [128, MIQ, T], BF16)    # exp(b)/8
    texk = nc.alloc_sbuf_tensor("texk", [128, MIQ, T], BF16)    # exp(-b)

    q_sb = nc.alloc_sbuf_tensor("q_sb", [128, MIQ, T], BF16)    # q then q~ (in place)
    k_sb = nc.alloc_sbuf_tensor("k_sb", [128, MIQ, T], BF16)    # k then k~
    v_sb = nc.alloc_sbuf_tensor("v_sb", [128, MIV, T], BF16)
    vT = nc.alloc_sbuf_tensor("vT", [128, NCH, VDC], BF16)      # time-major v per chunk
    gate_sb = nc.alloc_sbuf_tensor("gate_sb", [128, NCH, VDC], BF16)
    preA = nc.alloc_sbuf_tensor("preA", [128, T + 3], BF16)     # conv staging (pad 3)
    preB = nc.alloc_sbuf_tensor("preB", [128, T + 3], BF16)

    Sblk = nc.alloc_sbuf_tensor("Sblk", [128, G, 256], BF16)    # block-diag state
    qblk = nc.alloc_sbuf_tensor("qblk", [128, G, NCH, 256], BF16)  # block-diag q~
    khnat = nc.alloc_sbuf_tensor("khnat", [128, 256], BF16)     # time-major k~ chunk
    sqd = nc.alloc_sbuf_tensor("sqd", [128, 4, 128], BF16)      # Square trash
    ssq_all = nc.alloc_sbuf_tensor("ssq_all", [128, NCH * 4], F32)
    srms_all = nc.alloc_sbuf_tensor("srms_all", [128, NCH * 4], F32)
    rrms_all = nc.alloc_sbuf_tensor("rrms_all", [128, NCH * 4], F32)
    ogT = nc.alloc_sbuf_tensor("ogT", [128, MIV, T], BF16)
    stage = nc.alloc_sbuf_tensor("stage", [128, 4, 512], BF16)
    negln8 = nc.alloc_sbuf_tensor("negln8", [128, 1], F32)
    eps_col = nc.alloc_sbuf_tensor("eps_col", [128, 1], F32)

    with tile.TileContext(nc) as tc:
        with (
            tc.tile_pool(name="scr", bufs=4) as scr,
        ):
            # ---- phase 0: DMAs. src first, split across the 3 queues; tiny
            # gate-path weights lead on sync.
            dma_engs = [nc.sync, nc.scalar, nc.gpsimd]
            nc.sync.dma_start(out=bgn_sb[:], in_=bgn_d[:])
            nc.sync.dma_start(out=weff_sb[:], in_=weff_d[:].rearrange("p (kt m) -> p kt m", kt=8))
            for kt in range(8):
                dma_engs[kt % 3].dma_start(
                    out=srcT[:, kt, :],
                    in_=srcT_d[:, kt * 1024:(kt + 1) * 1024],
                )
            nc.sync.dma_start(out=wq_sb[:], in_=wq_d[:].rearrange("p (kt m) -> p kt m", kt=8))
            nc.sync.dma_start(out=maskc_sb[:], in_=maskc_d[:])

            nc.scalar.dma_start(out=ident[:], in_=ident_d[:])
            nc.scalar.dma_start(out=wk_sb[:], in_=wk_d[:].rearrange("p (kt m) -> p kt m", kt=8))
            nc.scalar.dma_start(out=dgall[:], in_=convdiag_d[:].rearrange("p (t m) -> p t m", t=NDIAG))
            nc.scalar.dma_start(out=triu4[:], in_=triu4_d[:])

            nc.gpsimd.dma_start(out=wv_sb[:], in_=wv_d[:].rearrange("p (kt m) -> p kt m", kt=8))
            nc.gpsimd.dma_start(out=wgate_sb[:], in_=wgate_d[:].rearrange("p (kt m) -> p kt m", kt=8))
            nc.gpsimd.dma_start(out=wo_sb[:], in_=wo_d[:].rearrange("p (h m) -> p h m", h=4))

            nc.vector.memset(ones_sb[:], 1.0)
            nc.vector.memset(preA[:, 0:3], 0.0)
            nc.vector.memset(preB[:, 0:3], 0.0)
            nc.vector.memset(negln8[:], -LN8)
            nc.vector.memset(eps_col[:], EPS)
            # warm the Exp/Ln activation table during the src DMA wait
            nc.scalar.activation(srms_all[:, 0:1], eps_col[:], AF.Exp)
            nc.gpsimd.memset(Sblk[:], 0.0)
            nc.gpsimd.memset(qblk[:], 0.0)

            with (
                tc.tile_pool(name="pp", bufs=6, space="PSUM") as pp,
            ):
                # ---- gk path: logit = src @ (Wg1 Wg2) + bg2, collapsed on
                # the host; the bias folds into the Exp (per-partition AP).
                # spT = softplus(-logit) = ln(1 + exp(-logit)).
                for mi in range(MIQ):
                    for nh in range(2):
                        p = pp.tile([128, 512], F32, name="pp_sp", tag="pp")
                        for kt in range(8):
                            nc.tensor.matmul(
                                p[:],
                                weff_sb[:, kt, mi * 128:(mi + 1) * 128],
                                srcT[:, kt, nh * 512:(nh + 1) * 512],
                                start=(kt == 0),
                                stop=(kt == 7),
                            )
                        nc.scalar.activation(
                            enx[:, mi, nh * 512:(nh + 1) * 512], p[:],
                            AF.Exp, scale=-1.0, bias=bgn_sb[:, mi:mi + 1],
                        )
                for mi in range(MIQ):
                    for nh in range(2):
                        sl = slice(nh * 512, (nh + 1) * 512)
                        nc.scalar.activation(
                            spT[:, mi, sl], enx[:, mi, sl], AF.Ln, bias=1.0
                        )
                # per-chunk inclusive cumsum; chunk-end decay factors; the
                # full-T exp factors for q~ and k~
                for mi in range(MIQ):
                    for c in range(NCH):
                        csl = slice(c * 128, (c + 1) * 128)
                        nc.vector.tensor_tensor_scan(
                            out=bsum[:, mi, csl],
                            data0=ones_sb[:],
                            data1=spT[:, mi, csl],
                            initial=0.0,
                            op0=OP.mult,
                            op1=OP.add,
                        )

                def conv_proj(w_sb, diag_base, dst, mi_count, pre_bufs):
                    """dst[:, mi, :] = silu(conv4(src @ W[:, mi-block]))^T."""
                    for mi in range(mi_count):
                        pre = pre_bufs[mi % 2]
                        for nh in range(2):
                            p = pp.tile([128, 512], F32, name="pp_prj", tag="pp")
                            for kt in range(8):
                                nc.tensor.matmul(
                                    p[:],
                                    w_sb[:, kt, mi * 128:(mi + 1) * 128],
                                    srcT[:, kt, nh * 512:(nh + 1) * 512],
                                    start=(kt == 0),
                                    stop=(kt == 7),
                                )
                            nc.vector.tensor_copy(
                                out=pre[:, 3 + nh * 512:3 + (nh + 1) * 512], in_=p[:]
                            )
                        for nh in range(2):
                            cp = pp.tile([128, 512], F32, name="pp_cnv", tag="pp")
                            for j in range(CONV):
                                nc.tensor.matmul(
                                    cp[:],
                                    dgall[:, (diag_base + mi) * CONV + j, :],
                                    pre[:, nh * 512 + j:nh * 512 + j + 512],
                                    start=(j == 0),
                                    stop=(j == 3),
                                )
                            nc.scalar.activation(
                                dst[:, mi, nh * 512:(nh + 1) * 512], cp[:], AF.Silu
                            )

                conv_proj(wq_sb, 0, q_sb, MIQ, (preA, preB))
                conv_proj(wk_sb, MIQ, k_sb, MIQ, (preA, preB))

                # gate-decay exps after the conv silus so the ACT queue is
                # not blocked on the DVE cumsum while conv PSUM piles up
                for mi in range(MIQ):
                    nc.scalar.activation(
                        Eall[:, mi, :], bsum[:, mi, 127::128], AF.Exp,
                        scale=-1.0 / GATE_NORM,
                    )
                    nc.scalar.activation(
                        texq[:, mi, :], bsum[:, mi, :], AF.Exp,
                        scale=-1.0 / GATE_NORM, bias=negln8[:],
                    )
                    nc.scalar.activation(
                        texk[:, mi, :], bsum[:, mi, :], AF.Exp,
                        scale=1.0 / GATE_NORM,
                    )

                # q~ = q * exp(b)/8 and k~ = k * exp(-b), in place (bf16 2x)
                for mi in range(MIQ):
                    for half in range(2):
                        hsl = slice(half * 512, (half + 1) * 512)
                        nc.vector.tensor_mul(
                            q_sb[:, mi, hsl], q_sb[:, mi, hsl], texq[:, mi, hsl]
                        )
                        nc.vector.tensor_mul(
                            k_sb[:, mi, hsl], k_sb[:, mi, hsl], texk[:, mi, hsl]
                        )
                # block-diagonal q~ for the per-head A~ matmuls (4 wide copies)
                for g in range(G):
                    nc.vector.tensor_copy(
                        out=qblk[0:64, g, :, 0:128],
                        in_=q_sb[0:64, g, :].rearrange("p (c x) -> p c x", c=NCH),
                    )
                    nc.vector.tensor_copy(
                        out=qblk[64:128, g, :, 128:256],
                        in_=q_sb[64:128, g, :].rearrange("p (c x) -> p c x", c=NCH),
                    )

                conv_proj(wv_sb, 2 * MIQ, v_sb, MIV, (preA, preB))


            # ---- chunk recurrence with the output projection pipelined in --
            with (
                tc.tile_pool(name="psm", bufs=3, space="PSUM") as psm,
                tc.tile_pool(name="po", bufs=2, space="PSUM") as po,
                tc.tile_pool(name="pout", bufs=3, space="PSUM") as pout,
            ):
                def gate_vt(mt):
                    """Gate projection (silu) and v transpose for chunk mt."""
                    p = psm.tile([128, 512], F32, name="pp_gate", tag="psm")
                    for kt in range(8):
                        nc.tensor.matmul(
                            p[:],
                            srcT[:, kt, mt * 128:(mt + 1) * 128],
                            wgate_sb[:, kt, :],
                            start=(kt == 0),
                            stop=(kt == 7),
                        )
                    nc.scalar.activation(gate_sb[:, mt, :], p[:], AF.Silu)
                    pv = po.tile([128, 512], F32, name="ps_v", tag="po")
                    vw = pv[:, 0:256].bitcast(BF16)   # [128, 512]
                    csl = slice(mt * 128, (mt + 1) * 128)
                    for h in range(4):
                        nc.tensor.matmul(
                            vw[:, h * 128:(h + 1) * 128], v_sb[:, h, csl],
                            ident[:], is_transpose=True,
                            start=(h == 0), stop=(h == 3),
                            skip_group_check=True,
                        )
                    nc.vector.tensor_scalar_mul(
                        vT[:, mt, :], vw[:], maskc_sb[:, mt:mt + 1]
                    )
                def emit_head(c):
                    """A~ (masked) and the k~ chunk transpose for chunk c."""
                    csl = slice(c * 128, (c + 1) * 128)
                    # A~ for all 4 heads in one bank (start=True only on the
                    # FIRST write to a bank: start resets the whole bank)
                    pa = psm.tile([128, 512], F32, name="pa", tag="psm")
                    for g in range(G):
                        nc.tensor.matmul(
                            pa[:, g * 256:(g + 1) * 256], k_sb[:, g, csl],
                            qblk[:, g, c, :], start=(g == 0), stop=(g == 1),
                            skip_group_check=True,
                        )
                    a_sb = scr.tile([128, 512], BF16, name="a_sb", tag="a_sb")
                    nc.vector.tensor_mul(a_sb[:], pa[:], triu4[:])
                    # k~ chunk transposes (both groups) in their own bank
                    misc = psm.tile([128, 512], F32, name="misc", tag="psm")
                    ktv = misc[:, 0:128].bitcast(BF16)     # [128, 256]
                    for g in range(G):
                        nc.tensor.matmul(
                            ktv[:, g * 128:(g + 1) * 128], k_sb[:, g, csl],
                            ident[:], is_transpose=True,
                            start=(g == 0), stop=(g == 1),
                            skip_group_check=True,
                        )
                    nc.vector.tensor_copy(out=khnat[:], in_=ktv[:])
                    return a_sb

                def emit_core(c, a_sb):
                    """o, state update, squares, swish gate, rrms for chunk c."""
                    csl = slice(c * 128, (c + 1) * 128)
                    ps_o = po.tile([128, 512], F32, name="ps_o", tag="po")
                    for h in range(4):
                        nc.tensor.matmul(
                            ps_o[:, h * 128:(h + 1) * 128],
                            a_sb[:, h * 128:(h + 1) * 128],
                            vT[:, c, h * 128:(h + 1) * 128],
                            start=(h == 0), stop=False, skip_group_check=True,
                        )
                    for g in range(G):
                        nc.tensor.matmul(
                            ps_o[:, g * 256:(g + 1) * 256], q_sb[:, g, csl],
                            Sblk[:, g, :], start=False, stop=(g == 1),
                            skip_group_check=True,
                        )
                    # state update term k~^T v (both groups one bank)
                    ps_s = pout.tile([128, 512], F32, name="ps_s", tag="pout")
                    for g in range(G):
                        nc.tensor.matmul(
                            ps_s[:, g * 256:(g + 1) * 256], khnat[:, g * 128:(g + 1) * 128],
                            vT[:, c, g * 256:(g + 1) * 256],
                            start=(g == 0), stop=(g == 1),
                            skip_group_check=True,
                        )
                    # S = e * (S + k~^T v): two strided block adds + one
                    # broadcast scale (o-inter above reads S first)
                    nc.vector.tensor_add(
                        Sblk[0:64, :, 0:128],
                        Sblk[0:64, :, 0:128],
                        ps_s[0:64, :].rearrange("p (g x) -> p g x", g=2)[:, :, 0:128],
                    )
                    nc.vector.tensor_add(
                        Sblk[64:128, :, 128:256],
                        Sblk[64:128, :, 128:256],
                        ps_s[64:128, :].rearrange("p (g x) -> p g x", g=2)[:, :, 128:256],
                    )
                    nc.vector.tensor_mul(
                        Sblk[:],
                        Sblk[:],
                        Eall[:, :, c:c + 1].to_broadcast((128, G, 256)),
                    )
                    # per-head sum of squares (Square lives in every table)
                    for h in range(4):
                        idx = c * 4 + h
                        nc.scalar.activation(
                            sqd[:, h, :], ps_o[:, h * 128:(h + 1) * 128],
                            AF.Square, accum_out=ssq_all[:, idx:idx + 1],
                        )
                    # swish gate multiplied in place (both groups, one op)
                    nc.vector.tensor_mul(
                        gate_sb[:, c, :], ps_o[:], gate_sb[:, c, :]
                    )
                    c4 = slice(c * 4, (c + 1) * 4)
                    nc.scalar.activation(
                        srms_all[:, c4], ssq_all[:, c4], AF.Sqrt,
                        scale=1.0 / DV, bias=eps_col[:],
                    )
                    nc.vector.reciprocal(rrms_all[:, c4], srms_all[:, c4])
                    nc.vector.tensor_mul(
                        gate_sb[:, c, :].rearrange("p (h x) -> p h x", h=4),
                        gate_sb[:, c, :].rearrange("p (h x) -> p h x", h=4),
                        rrms_all[:, c4, None].to_broadcast((128, 4, 128)),
                    )

                def emit_tail(c):
                    """Transpose the gated output and project through Wo."""
                    csl = slice(c * 128, (c + 1) * 128)
                    ogm = psm.tile([128, 512], F32, name="ogm", tag="psm")
                    ogv = ogm[:, 0:256].bitcast(BF16)      # [128, 512]
                    for h in range(4):
                        nc.tensor.matmul(
                            ogv[:, h * 128:(h + 1) * 128],
                            gate_sb[:, c, h * 128:(h + 1) * 128],
                            ident[:], is_transpose=True,
                            start=(h == 0), stop=(h == 3),
                            skip_group_check=True,
                        )
                    nc.vector.tensor_copy(
                        out=ogT[:, :, csl],
                        in_=ogv[:].rearrange("p (a b) -> p a b", a=4),
                    )
                    for nh in range(2):
                        p = pout.tile([128, 512], F32, name="p_out", tag="pout")
                        for h in range(4):
                            nc.tensor.matmul(
                                p[:],
                                ogT[:, h, csl],
                                wo_sb[:, h, nh * 512:(nh + 1) * 512],
                                start=(h == 0),
                                stop=(h == 3),
                            )
                        st = stage[:, 2 * (c % 2) + nh, :]
                        nc.scalar.copy(out=st, in_=p[:])
                        dma_eng = nc.sync if nh == 0 else nc.gpsimd
                        dma_eng.dma_start(
                            out=out_d[c * 128:(c + 1) * 128,
                                      nh * 512:(nh + 1) * 512],
                            in_=st,
                        )

                # software-pipelined: the gate projection and v transpose
                # for chunk c+1 plus chunk c's head are queued before chunk
                # c-1's tail, so the PE always has runnable matmuls while
                # DVE/ACT finish the previous chunk's gate/rrms chain
                gate_vt(0)
                gate_vt(1)
                a_prev = emit_head(0)
                emit_core(0, a_prev)
                for c in range(1, NCH):
                    if c + 1 < NCH:
                        gate_vt(c + 1)
                    a_prev = emit_head(c)
                    if c >= 2:
                        emit_tail(c - 2)
                    emit_core(c, a_prev)
                emit_tail(NCH - 2)
                emit_tail(NCH - 1)

    nc.compile()
    return nc


_NC_CACHE = None


def _get_program():
    global _NC_CACHE
    if _NC_CACHE is None:
        _NC_CACHE = build_program()
    return _NC_CACHE


def _packT(a, kt):
    """[R, M] -> [128, kt*M] bf16 with R = kt*128 split partition-major."""
    r, m = a.shape
    assert r == kt * 128
    return np.ascontiguousarray(
        a.reshape(kt, 128, m).transpose(1, 0, 2).reshape(128, kt * m)
    ).astype(BF)


def shard_inputs(
    src, valid_mask, Wq, Wk, Wv, conv_q_w, conv_k_w, conv_v_w,
    Wg1, Wg2, bg2, Wgate, rms_w, Wo,
):
    """Build the 8 per-core input maps (everything pre-packed, bf16)."""
    f = np.float32
    src = np.asarray(src, f)
    valid_mask = np.asarray(valid_mask)
    in_maps = []
    wo_scaled = np.asarray(Wo, f) * np.tile(np.asarray(rms_w, f), VD // DV)[:, None]
    for core in range(NCORES):
        b, hg = core // 2, core % 2
        qs = slice(hg * KDC, (hg + 1) * KDC)
        vs = slice(hg * VDC, (hg + 1) * VDC)
        weff = np.asarray(Wg1, f) @ np.asarray(Wg2, f)[:, qs]   # [D, KDC]
        bgn = -np.asarray(bg2, f)[qs].reshape(MIQ, 128).T        # [128, MIQ]

        # the 32 conv diag matrices, side by side: q0,q1,k0,k1,v0..v3 x 4 taps
        conv_diag = np.zeros((128, NDIAG * 128), f)
        ti = 0
        for w, sel, n in ((conv_q_w, qs, MIQ), (conv_k_w, qs, MIQ),
                          (conv_v_w, vs, MIV)):
            wa = np.asarray(w, f)[sel]
            for i in range(n):
                for j in range(CONV):
                    d = (ti * CONV + j) * 128
                    conv_diag[np.arange(128), d + np.arange(128)] = \
                        wa[i * 128:(i + 1) * 128, j]
                ti += 1

        in_maps.append({
            "srcT_in": _packT(src[b].T, 8),
            "wq": _packT(np.asarray(Wq, f)[:, qs], 8),
            "wk": _packT(np.asarray(Wk, f)[:, qs], 8),
            "wv": _packT(np.asarray(Wv, f)[:, vs], 8),
            "wgate": _packT(np.asarray(Wgate, f)[:, vs], 8),
            "weff": _packT(weff, 8),
            "bgn": np.ascontiguousarray(bgn),
            "wo": _packT(wo_scaled[vs, :], 4),
            "convdiag": conv_diag.astype(BF),
            "maskc": np.ascontiguousarray(
                valid_mask[b].astype(f).reshape(NCH, 128).T
            ),
        })
    return in_maps


def kernel(**inputs):
    nc = _get_program()
    in_maps = shard_inputs(**inputs)
    res = run_bass_kernel_spmd(nc, in_maps, list(range(NCORES)))
    out = np.zeros((B, T, D), np.float32)
    for core in range(NCORES):
        out[core // 2] += res.results[core]["out"].astype(np.float32)
    return out


if __name__ == "__main__":
    prog = _get_program()
    print("program built OK")


# revision 33
# speedup vs baseline: 1.0332x; 1.0332x over previous
"""Gated Linear Attention forward on 8 Trainium2 NeuronCores (Bass/Tile).

Problem: B=4, T=1024, D=1024, H=8, DK=64, DV=128, conv4 on q/k/v, low-rank
log-sigmoid forget gate, recurrent scan, RMS-norm + swish output gate, out proj.

Sharding: core = 2*b + hg  (b = batch, hg = half of the heads).
Each core computes its batch's tokens for 4 heads end-to-end and a partial
output projection (Wo row-block); the host sums the two partials per batch.

On-device algorithm: chunked-parallel GLA with chunk C=128.
Per chunk (local inclusive cumsum b of the log-gates):
  q~ = q * exp(b)/8,  k~ = k * exp(-b)
  A~[s,t] = sum_kk k~[s] q~[t]   masked to s<=t
  o = A~^T v (intra) + q~ @ S (inter), accumulated in one PSUM tile
  S' = diag(exp(b_C)) (S + k~^T v)

v3 engine notes (informed by HW traces):
- All matmul operands bf16 (PSUM fp32); gk/cumsum fp32. Host pre-packs
  every input into [128, ...] partition-major bf16 slabs (incl. the conv
  diag matrices) so each is one simple 2D HWDGE DMA; src lands first split
  across the 3 DMA-capable queues.
- The GpSimd engine only does memsets + output DMA (its software tensor
  ops measure ~2.4us each - never on the critical path).
- ACT table switches are expensive; the chunk phase uses only
  Copy/Square/Sqrt (one table): rrms = DVE-reciprocal(ACT-Sqrt(ms+eps)).
- Small per-(c,g) ops are batched full-width: one 512-wide PSUM bank per
  step holds both head groups (A~, o, k^T v), so DVE/ACT run 1 wide op
  instead of 2-4 narrow ones.
- The output projection is pipelined per chunk inside the recurrence loop
  so the PE never drains; output DMA (bf16) overlaps the chunk compute.
"""

import numpy as np
import ml_dtypes

import concourse.bass as bass
import concourse.mybir as mybir
import concourse.tile as tile
from concourse import bacc
from concourse.bass_utils import run_bass_kernel_spmd

F32 = mybir.dt.float32
BF16 = mybir.dt.bfloat16
AF = mybir.ActivationFunctionType
OP = mybir.AluOpType

# problem constants (hardcoded per the task contract)
B, T, D, H = 4, 1024, 1024, 8
KD, VD = 512, 1024
DK, DV = 64, 128
CONV = 4
GATE_NORM = 16.0
EPS = 1e-5
LN8 = float(np.log(8.0))

# per-core shapes
KDC, VDC = 256, 512          # q/k and v/gate channels per core
MIQ, MIV = 2, 4              # 128-wide channel tiles for q/k and v
C, NCH = 128, 8              # chunk length, number of chunks
G = 2                        # head groups of 2 heads (128 chans) per core
NCORES = 8
NDIAG = (2 * MIQ + MIV) * CONV   # 32 diag matrices

BF = ml_dtypes.bfloat16


def build_program():
    nc = bacc.Bacc("TRN2", target_bir_lowering=False, debug=False)

    # ---- DRAM I/O (all pre-packed partition-major on the host) -------------
    srcT_d = nc.dram_tensor("srcT_in", [128, 8 * T], BF16, kind="ExternalInput")
    wq_d = nc.dram_tensor("wq", [128, 8 * KDC], BF16, kind="ExternalInput")
    wk_d = nc.dram_tensor("wk", [128, 8 * KDC], BF16, kind="ExternalInput")
    wv_d = nc.dram_tensor("wv", [128, 8 * VDC], BF16, kind="ExternalInput")
    wgate_d = nc.dram_tensor("wgate", [128, 8 * VDC], BF16, kind="ExternalInput")
    wg1_d = nc.dram_tensor("wg1", [128, 8 * 16], BF16, kind="ExternalInput")
    wg2b_d = nc.dram_tensor("wg2b", [17, KDC], BF16, kind="ExternalInput")
    wo_d = nc.dram_tensor("wo", [128, 4 * D], BF16, kind="ExternalInput")
    convdiag_d = nc.dram_tensor("convdiag", [128, NDIAG * 128], BF16,
                                kind="ExternalInput")
    maskc_d = nc.dram_tensor("maskc", [128, NCH], F32, kind="ExternalInput")
    out_d = nc.dram_tensor("out", [T, D], BF16, kind="ExternalOutput")

    ident_np = np.eye(128, dtype=np.float32).astype(BF)
    u = np.triu(np.ones((128, 128), np.float32))  # U[s,t] = 1 iff s <= t
    triu4_np = np.concatenate([u, u, u, u], axis=1).astype(BF)
    ident_d = nc.inline_tensor(ident_np, "ident_c")
    triu4_d = nc.inline_tensor(triu4_np, "triu4_c")

    # ---- static SBUF -------------------------------------------------------
    srcT = nc.alloc_sbuf_tensor("srcT", [128, 8, T], BF16)      # src^T, d-major
    wq_sb = nc.alloc_sbuf_tensor("wq_sb", [128, 8, KDC], BF16)
    wk_sb = nc.alloc_sbuf_tensor("wk_sb", [128, 8, KDC], BF16)
    wv_sb = nc.alloc_sbuf_tensor("wv_sb", [128, 8, VDC], BF16)
    wgate_sb = nc.alloc_sbuf_tensor("wgate_sb", [128, 8, VDC], BF16)
    wg1_sb = nc.alloc_sbuf_tensor("wg1_sb", [128, 8, 16], BF16)
    wg2b_sb = nc.alloc_sbuf_tensor("wg2b_sb", [17, KDC], BF16)
    wo_sb = nc.alloc_sbuf_tensor("wo_sb", [128, 4, D], BF16)
    dgall = nc.alloc_sbuf_tensor("dgall", [128, NDIAG, 128], BF16)
    maskc_sb = nc.alloc_sbuf_tensor("maskc_sb", [128, NCH], F32)
    ident = nc.alloc_sbuf_tensor("ident", [128, 128], BF16)
    triu4 = nc.alloc_sbuf_tensor("triu4", [128, 512], BF16)
    ones_sb = nc.alloc_sbuf_tensor("ones_sb", [128, 128], F32)

    xgT = nc.alloc_sbuf_tensor("xgT", [17, T], BF16)            # (src@Wg1)^T + ones row
    enx = nc.alloc_sbuf_tensor("enx", [128, MIQ, T], F32)       # exp(-logit)
    spT = nc.alloc_sbuf_tensor("spT", [128, MIQ, T], F32)       # softplus(-gk_logit)
    bsum = nc.alloc_sbuf_tensor("bsum", [128, MIQ, T], F32)     # per-chunk cumsum
    Eall = nc.alloc_sbuf_tensor("Eall", [128, MIQ, NCH], F32)   # exp(b_C) per chunk
    texq = nc.alloc_sbuf_tensor("texq", [128, MIQ, T], BF16)    # exp(b)/8
    texk = nc.alloc_sbuf_tensor("texk", [128, MIQ, T], BF16)    # exp(-b)

    q_sb = nc.alloc_sbuf_tensor("q_sb", [128, MIQ, T], BF16)    # q then q~ (in place)
    k_sb = nc.alloc_sbuf_tensor("k_sb", [128, MIQ, T], BF16)    # k then k~
    v_sb = nc.alloc_sbuf_tensor("v_sb", [128, MIV, T], BF16)
    vT = nc.alloc_sbuf_tensor("vT", [128, NCH, VDC], BF16)      # time-major v per chunk
    gate_sb = nc.alloc_sbuf_tensor("gate_sb", [128, NCH, VDC], BF16)
    preA = nc.alloc_sbuf_tensor("preA", [128, T + 3], BF16)     # conv staging (pad 3)
    preB = nc.alloc_sbuf_tensor("preB", [128, T + 3], BF16)

    Sblk = nc.alloc_sbuf_tensor("Sblk", [128, G, 256], BF16)    # block-diag state
    qblk = nc.alloc_sbuf_tensor("qblk", [128, G, NCH, 256], BF16)  # block-diag q~
    khnat = nc.alloc_sbuf_tensor("khnat", [128, 256], BF16)     # time-major k~ chunk
    sqd = nc.alloc_sbuf_tensor("sqd", [128, 4, 128], BF16)      # Square trash
    ssq_all = nc.alloc_sbuf_tensor("ssq_all", [128, NCH * 4], F32)
    srms_all = nc.alloc_sbuf_tensor("srms_all", [128, NCH * 4], F32)
    rrms_all = nc.alloc_sbuf_tensor("rrms_all", [128, NCH * 4], F32)
    ogT = nc.alloc_sbuf_tensor("ogT", [128, MIV, T], BF16)
    stage = nc.alloc_sbuf_tensor("stage", [128, 4, 512], BF16)
    negln8 = nc.alloc_sbuf_tensor("negln8", [128, 1], F32)
    eps_col = nc.alloc_sbuf_tensor("eps_col", [128, 1], F32)

    with tile.TileContext(nc) as tc:
        with (
            tc.tile_pool(name="scr", bufs=4) as scr,
        ):
            # ---- phase 0: DMAs. src first, split across the 3 queues; tiny
            # gate-path weights lead on sync.
            dma_engs = [nc.sync, nc.scalar, nc.gpsimd]
            nc.sync.dma_start(out=wg1_sb[:], in_=wg1_d[:].rearrange("p (kt m) -> p kt m", kt=8))
            nc.sync.dma_start(out=wg2b_sb[:], in_=wg2b_d[:])
            for kt in range(8):
                dma_engs[kt % 3].dma_start(
                    out=srcT[:, kt, :],
                    in_=srcT_d[:, kt * 1024:(kt + 1) * 1024],
                )
            nc.sync.dma_start(out=wq_sb[:], in_=wq_d[:].rearrange("p (kt m) -> p kt m", kt=8))
            nc.sync.dma_start(out=maskc_sb[:], in_=maskc_d[:])

            nc.scalar.dma_start(out=ident[:], in_=ident_d[:])
            nc.scalar.dma_start(out=wk_sb[:], in_=wk_d[:].rearrange("p (kt m) -> p kt m", kt=8))
            nc.scalar.dma_start(out=dgall[:], in_=convdiag_d[:].rearrange("p (t m) -> p t m", t=NDIAG))
            nc.scalar.dma_start(out=triu4[:], in_=triu4_d[:])

            nc.gpsimd.dma_start(out=wv_sb[:], in_=wv_d[:].rearrange("p (kt m) -> p kt m", kt=8))
            nc.gpsimd.dma_start(out=wgate_sb[:], in_=wgate_d[:].rearrange("p (kt m) -> p kt m", kt=8))
            nc.gpsimd.dma_start(out=wo_sb[:], in_=wo_d[:].rearrange("p (h m) -> p h m", h=4))

            nc.vector.memset(ones_sb[:], 1.0)
            nc.vector.memset(xgT[:], 1.0)   # row 16 = bias row; 0..15 overwritten
            nc.vector.memset(preA[:, 0:3], 0.0)
            nc.vector.memset(preB[:, 0:3], 0.0)
            nc.vector.memset(negln8[:], -LN8)
            nc.vector.memset(eps_col[:], EPS)
            # warm the Exp/Ln activation table during the src DMA wait
            nc.scalar.activation(srms_all[:, 0:1], eps_col[:], AF.Exp)
            nc.gpsimd.memset(Sblk[:], 0.0)
            nc.gpsimd.memset(qblk[:], 0.0)

            with (
                tc.tile_pool(name="pp", bufs=6, space="PSUM") as pp,
            ):
                # ---- gk path: xg^T = (src @ Wg1)^T with appended ones row
                for nh in range(2):
                    p = pp.tile([128, 512], F32, name="pp_xg", tag="pp")
                    for kt in range(8):
                        nc.tensor.matmul(
                            p[0:16, :],
                            wg1_sb[:, kt, :],
                            srcT[:, kt, nh * 512:(nh + 1) * 512],
                            start=(kt == 0),
                            stop=(kt == 7),
                        )
                    nc.vector.tensor_copy(
                        out=xgT[0:16, nh * 512:(nh + 1) * 512], in_=p[0:16, :]
                    )
                # spT = softplus(-(xg @ Wg2 + bg2)) = ln(1 + exp(-logit));
                # all Exps before all Lns (one table switch).
                for mi in range(MIQ):
                    for nh in range(2):
                        p = pp.tile([128, 512], F32, name="pp_sp", tag="pp")
                        nc.tensor.matmul(
                            p[:],
                            wg2b_sb[:, mi * 128:(mi + 1) * 128],
                            xgT[:, nh * 512:(nh + 1) * 512],
                            start=True,
                            stop=True,
                        )
                        nc.scalar.activation(
                            enx[:, mi, nh * 512:(nh + 1) * 512], p[:],
                            AF.Exp, scale=-1.0,
                        )
                for mi in range(MIQ):
                    for nh in range(2):
                        sl = slice(nh * 512, (nh + 1) * 512)
                        nc.scalar.activation(
                            spT[:, mi, sl], enx[:, mi, sl], AF.Ln, bias=1.0
                        )
                # per-chunk inclusive cumsum; chunk-end decay factors; the
                # full-T exp factors for q~ and k~
                for mi in range(MIQ):
                    for c in range(NCH):
                        csl = slice(c * 128, (c + 1) * 128)
                        nc.vector.tensor_tensor_scan(
                            out=bsum[:, mi, csl],
                            data0=ones_sb[:],
                            data1=spT[:, mi, csl],
                            initial=0.0,
                            op0=OP.mult,
                            op1=OP.add,
                        )

                def conv_proj(w_sb, diag_base, dst, mi_count, pre_bufs):
                    """dst[:, mi, :] = silu(conv4(src @ W[:, mi-block]))^T."""
                    for mi in range(mi_count):
                        pre = pre_bufs[mi % 2]
                        for nh in range(2):
                            p = pp.tile([128, 512], F32, name="pp_prj", tag="pp")
                            for kt in range(8):
                                nc.tensor.matmul(
                                    p[:],
                                    w_sb[:, kt, mi * 128:(mi + 1) * 128],
                                    srcT[:, kt, nh * 512:(nh + 1) * 512],
                                    start=(kt == 0),
                                    stop=(kt == 7),
                                )
                            nc.vector.tensor_copy(
                                out=pre[:, 3 + nh * 512:3 + (nh + 1) * 512], in_=p[:]
                            )
                        for nh in range(2):
                            cp = pp.tile([128, 512], F32, name="pp_cnv", tag="pp")
                            for j in range(CONV):
                                nc.tensor.matmul(
                                    cp[:],
                                    dgall[:, (diag_base + mi) * CONV + j, :],
                                    pre[:, nh * 512 + j:nh * 512 + j + 512],
                                    start=(j == 0),
                                    stop=(j == 3),
                                )
                            nc.scalar.activation(
                                dst[:, mi, nh * 512:(nh + 1) * 512], cp[:], AF.Silu
                            )

                conv_proj(wq_sb, 0, q_sb, MIQ, (preA, preB))
                conv_proj(wk_sb, MIQ, k_sb, MIQ, (preA, preB))

                # gate-decay exps after the conv silus so the ACT queue is
                # not blocked on the DVE cumsum while conv PSUM piles up
                for mi in range(MIQ):
                    nc.scalar.activation(
                        Eall[:, mi, :], bsum[:, mi, 127::128], AF.Exp,
                        scale=-1.0 / GATE_NORM,
                    )
                    nc.scalar.activation(
                        texq[:, mi, :], bsum[:, mi, :], AF.Exp,
                        scale=-1.0 / GATE_NORM, bias=negln8[:],
                    )
                    nc.scalar.activation(
                        texk[:, mi, :], bsum[:, mi, :], AF.Exp,
                        scale=1.0 / GATE_NORM,
                    )

                # q~ = q * exp(b)/8 and k~ = k * exp(-b), in place (bf16 2x)
                for mi in range(MIQ):
                    for half in range(2):
                        hsl = slice(half * 512, (half + 1) * 512)
                        nc.vector.tensor_mul(
                            q_sb[:, mi, hsl], q_sb[:, mi, hsl], texq[:, mi, hsl]
                        )
                        nc.vector.tensor_mul(
                            k_sb[:, mi, hsl], k_sb[:, mi, hsl], texk[:, mi, hsl]
                        )
                # block-diagonal q~ for the per-head A~ matmuls (4 wide copies)
                for g in range(G):
                    nc.vector.tensor_copy(
                        out=qblk[0:64, g, :, 0:128],
                        in_=q_sb[0:64, g, :].rearrange("p (c x) -> p c x", c=NCH),
                    )
                    nc.vector.tensor_copy(
                        out=qblk[64:128, g, :, 128:256],
                        in_=q_sb[64:128, g, :].rearrange("p (c x) -> p c x", c=NCH),
                    )

                conv_proj(wv_sb, 2 * MIQ, v_sb, MIV, (preA, preB))


            # ---- chunk recurrence with the output projection pipelined in --
            with (
                tc.tile_pool(name="psm", bufs=3, space="PSUM") as psm,
                tc.tile_pool(name="po", bufs=2, space="PSUM") as po,
                tc.tile_pool(name="pout", bufs=3, space="PSUM") as pout,
            ):
                def gate_vt(mt):
                    """Gate projection (silu) and v transpose for chunk mt."""
                    p = psm.tile([128, 512], F32, name="pp_gate", tag="psm")
                    for kt in range(8):
                        nc.tensor.matmul(
                            p[:],
                            srcT[:, kt, mt * 128:(mt + 1) * 128],
                            wgate_sb[:, kt, :],
                            start=(kt == 0),
                            stop=(kt == 7),
                        )
                    nc.scalar.activation(gate_sb[:, mt, :], p[:], AF.Silu)
                    pv = po.tile([128, 512], F32, name="ps_v", tag="po")
                    vw = pv[:, 0:256].bitcast(BF16)   # [128, 512]
                    csl = slice(mt * 128, (mt + 1) * 128)
                    for h in range(4):
                        nc.tensor.matmul(
                            vw[:, h * 128:(h + 1) * 128], v_sb[:, h, csl],
                            ident[:], is_transpose=True,
                            start=(h == 0), stop=(h == 3),
                            skip_group_check=True,
                        )
                    nc.vector.tensor_scalar_mul(
                        vT[:, mt, :], vw[:], maskc_sb[:, mt:mt + 1]
                    )
                def emit_head(c):
                    """A~ (masked) and the k~ chunk transpose for chunk c."""
                    csl = slice(c * 128, (c + 1) * 128)
                    # A~ for all 4 heads in one bank (start=True only on the
                    # FIRST write to a bank: start resets the whole bank)
                    pa = psm.tile([128, 512], F32, name="pa", tag="psm")
                    for g in range(G):
                        nc.tensor.matmul(
                            pa[:, g * 256:(g + 1) * 256], k_sb[:, g, csl],
                            qblk[:, g, c, :], start=(g == 0), stop=(g == 1),
                            skip_group_check=True,
                        )
                    a_sb = scr.tile([128, 512], BF16, name="a_sb", tag="a_sb")
                    nc.vector.tensor_mul(a_sb[:], pa[:], triu4[:])
                    # k~ chunk transposes (both groups) in their own bank
                    misc = psm.tile([128, 512], F32, name="misc", tag="psm")
                    ktv = misc[:, 0:128].bitcast(BF16)     # [128, 256]
                    for g in range(G):
                        nc.tensor.matmul(
                            ktv[:, g * 128:(g + 1) * 128], k_sb[:, g, csl],
                            ident[:], is_transpose=True,
                            start=(g == 0), stop=(g == 1),
                            skip_group_check=True,
                        )
                    nc.vector.tensor_copy(out=khnat[:], in_=ktv[:])
                    return a_sb

                def emit_core(c, a_sb):
                    """o, state update, squares, swish gate, rrms for chunk c."""
                    csl = slice(c * 128, (c + 1) * 128)
                    ps_o = po.tile([128, 512], F32, name="ps_o", tag="po")
                    for h in range(4):
                        nc.tensor.matmul(
                            ps_o[:, h * 128:(h + 1) * 128],
                            a_sb[:, h * 128:(h + 1) * 128],
                            vT[:, c, h * 128:(h + 1) * 128],
                            start=(h == 0), stop=False, skip_group_check=True,
                        )
                    for g in range(G):
                        nc.tensor.matmul(
                            ps_o[:, g * 256:(g + 1) * 256], q_sb[:, g, csl],
                            Sblk[:, g, :], start=False, stop=(g == 1),
                            skip_group_check=True,
                        )
                    # state update term k~^T v (both groups one bank)
                    ps_s = pout.tile([128, 512], F32, name="ps_s", tag="pout")
                    for g in range(G):
                        nc.tensor.matmul(
                            ps_s[:, g * 256:(g + 1) * 256], khnat[:, g * 128:(g + 1) * 128],
                            vT[:, c, g * 256:(g + 1) * 256],
                            start=(g == 0), stop=(g == 1),
                            skip_group_check=True,
                        )
                    # S = e * (S + k~^T v): two strided block adds + one
                    # broadcast scale (o-inter above reads S first)
                    nc.vector.tensor_add(
                        Sblk[0:64, :, 0:128],
                        Sblk[0:64, :, 0:128],
                        ps_s[0:64, :].rearrange("p (g x) -> p g x", g=2)[:, :, 0:128],
                    )
                    nc.vector.tensor_add(
                        Sblk[64:128, :, 128:256],
                        Sblk[64:128, :, 128:256],
                        ps_s[64:128, :].rearrange("p (g x) -> p g x", g=2)[:, :, 128:256],
                    )
                    nc.vector.tensor_mul(
                        Sblk[:],
                        Sblk[:],
                        Eall[:, :, c:c + 1].to_broadcast((128, G, 256)),
                    )
                    # per-head sum of squares (Square lives in every table)
                    for h in range(4):
                        idx = c * 4 + h
                        nc.scalar.activation(
                            sqd[:, h, :], ps_o[:, h * 128:(h + 1) * 128],
                            AF.Square, accum_out=ssq_all[:, idx:idx + 1],
                        )
                    # swish gate multiplied in place (both groups, one op)
                    nc.vector.tensor_mul(
                        gate_sb[:, c, :], ps_o[:], gate_sb[:, c, :]
                    )
                    c4 = slice(c * 4, (c + 1) * 4)
                    nc.scalar.activation(
                        srms_all[:, c4], ssq_all[:, c4], AF.Sqrt,
                        scale=1.0 / DV, bias=eps_col[:],
                    )
                    nc.vector.reciprocal(rrms_all[:, c4], srms_all[:, c4])
                    nc.vector.tensor_mul(
                        gate_sb[:, c, :].rearrange("p (h x) -> p h x", h=4),
                        gate_sb[:, c, :].rearrange("p (h x) -> p h x", h=4),
                        rrms_all[:, c4, None].to_broadcast((128, 4, 128)),
                    )

                def emit_tail(c):
                    """Transpose the gated output and project through Wo."""
                    csl = slice(c * 128, (c + 1) * 128)
                    ogm = psm.tile([128, 512], F32, name="ogm", tag="psm")
                    ogv = ogm[:, 0:256].bitcast(BF16)      # [128, 512]
                    for h in range(4):
                        nc.tensor.matmul(
                            ogv[:, h * 128:(h + 1) * 128],
                            gate_sb[:, c, h * 128:(h + 1) * 128],
                            ident[:], is_transpose=True,
                            start=(h == 0), stop=(h == 3),
                            skip_group_check=True,
                        )
                    nc.vector.tensor_copy(
                        out=ogT[:, :, csl],
                        in_=ogv[:].rearrange("p (a b) -> p a b", a=4),
                    )
                    for nh in range(2):
                        p = pout.tile([128, 512], F32, name="p_out", tag="pout")
                        for h in range(4):
                            nc.tensor.matmul(
                                p[:],
                                ogT[:, h, csl],
                                wo_sb[:, h, nh * 512:(nh + 1) * 512],
                                start=(h == 0),
                                stop=(h == 3),
                            )
                        st = stage[:, 2 * (c % 2) + nh, :]
                        nc.scalar.copy(out=st, in_=p[:])
                        dma_eng = nc.sync if nh == 0 else nc.gpsimd
                        dma_eng.dma_start(
                            out=out_d[c * 128:(c + 1) * 128,
                                      nh * 512:(nh + 1) * 512],
                            in_=st,
                        )

                # software-pipelined: the gate projection and v transpose
                # for chunk c+1 plus chunk c's head are queued before chunk
                # c-1's tail, so the PE always has runnable matmuls while
                # DVE/ACT finish the previous chunk's gate/rrms chain
                gate_vt(0)
                gate_vt(1)
                a_prev = emit_head(0)
                emit_core(0, a_prev)
                for c in range(1, NCH):
                    if c + 1 < NCH:
                        gate_vt(c + 1)
                    a_prev = emit_head(c)
                    if c >= 2:
                        emit_tail(c - 2)
                    emit_core(c, a_prev)
                emit_tail(NCH - 2)
                emit_tail(NCH - 1)

    nc.compile()
    return nc


_NC_CACHE = None


def _get_program():
    global _NC_CACHE
    if _NC_CACHE is None:
        _NC_CACHE = build_program()
    return _NC_CACHE


def _packT(a, kt):
    """[R, M] -> [128, kt*M] bf16 with R = kt*128 split partition-major."""
    r, m = a.shape
    assert r == kt * 128
    return np.ascontiguousarray(
        a.reshape(kt, 128, m).transpose(1, 0, 2).reshape(128, kt * m)
    ).astype(BF)


def shard_inputs(
    src, valid_mask, Wq, Wk, Wv, conv_q_w, conv_k_w, conv_v_w,
    Wg1, Wg2, bg2, Wgate, rms_w, Wo,
):
    """Build the 8 per-core input maps (everything pre-packed, bf16)."""
    f = np.float32
    src = np.asarray(src, f)
    valid_mask = np.asarray(valid_mask)
    in_maps = []
    wo_scaled = np.asarray(Wo, f) * np.tile(np.asarray(rms_w, f), VD // DV)[:, None]
    for core in range(NCORES):
        b, hg = core // 2, core % 2
        qs = slice(hg * KDC, (hg + 1) * KDC)
        vs = slice(hg * VDC, (hg + 1) * VDC)
        wg2b = np.concatenate(
            [np.asarray(Wg2, f)[:, qs], np.asarray(bg2, f)[None, qs]], axis=0
        )

        # the 32 conv diag matrices, side by side: q0,q1,k0,k1,v0..v3 x 4 taps
        conv_diag = np.zeros((128, NDIAG * 128), f)
        ti = 0
        for w, sel, n in ((conv_q_w, qs, MIQ), (conv_k_w, qs, MIQ),
                          (conv_v_w, vs, MIV)):
            wa = np.asarray(w, f)[sel]
            for i in range(n):
                for j in range(CONV):
                    d = (ti * CONV + j) * 128
                    conv_diag[np.arange(128), d + np.arange(128)] = \
                        wa[i * 128:(i + 1) * 128, j]
                ti += 1

        in_maps.append({
            "srcT_in": _packT(src[b].T, 8),
            "wq": _packT(np.asarray(Wq, f)[:, qs], 8),
            "wk": _packT(np.asarray(Wk, f)[:, qs], 8),
            "wv": _packT(np.asarray(Wv, f)[:, vs], 8),
            "wgate": _packT(np.asarray(Wgate, f)[:, vs], 8),
            "wg1": _packT(np.asarray(Wg1, f), 8),
            "wg2b": np.ascontiguousarray(wg2b).astype(BF),
            "wo": _packT(wo_scaled[vs, :], 4),
            "convdiag": conv_diag.astype(BF),
            "maskc": np.ascontiguousarray(
                valid_mask[b].astype(f).reshape(NCH, 128).T
            ),
        })
    return in_maps


def kernel(**inputs):
    nc = _get_program()
    in_maps = shard_inputs(**inputs)
    res = run_bass_kernel_spmd(nc, in_maps, list(range(NCORES)))
    out = np.zeros((B, T, D), np.float32)
    for core in range(NCORES):
        out[core // 2] += res.results[core]["out"].astype(np.float32)
    return out


if __name__ == "__main__":
    prog = _get_program()
    print("program built OK")
